# revision 35
# baseline (speedup 1.0000x reference)
"""EuclideanCodebook (vq_codebook) Trainium2 Bass kernel.

Sharding: frames (N axis) split 8 ways; codebook replicated.

Per core (nf frames):
  prologue: embT via PE transpose; Wt = 2*embT; neg_enorm row/cols.
  For each frame f:
    DMA x_nat [128, 4, 64]; 4x PE transpose -> xT_ps [65, 512] PSUM rows 0..63
    ACT Square(xT_ps) -> sq_sb; PE (-1s) @ sq_sb -> xT_ps row 64 (= -xnorm)
    ACT Copy xT_ps[0:65] -> xT_sb
    for t in 4 j-tiles:
      PE matmul: g[j, m] = 2*e[j,:].x[m,:] - xnorm[m]   (K=65)
      DVE ttscan: r[j, m] = running max over m
      DVE: vmax_s[j] = r[:, M-1] + neg_enorm[j]  -> V_t[:, f]
      ACT Sign+accum: m*[j] = #{m : r[j,m] < r[j,M-1]} -> MI_t[:, f]
  Stage 2: PE-transpose V/MI to [frame, j], scan+count over j -> argmin_curr,
  indirect-DMA gathers for residual = x[f, sel] - embed[j*].
"""

from contextlib import ExitStack

import numpy as np

NCORES = 8
N, M, C, CS = 512, 512, 64, 512
NF = N // NCORES  # frames per core
JT = CS // 128    # j tiles

FLT_MIN = -3.0e38

_cache = {}


def _emit(tc, nf, x_d, e_d, am_d, ac_d, res_d):
    import concourse.bass as bass
    import concourse.mybir as mybir
    from concourse.masks import make_identity

    nc = tc.nc
    f32 = mybir.dt.float32
    i32 = mybir.dt.int32
    u16 = mybir.dt.uint16
    AF = mybir.ActivationFunctionType

    with ExitStack() as ctx:
        persist = ctx.enter_context(tc.tile_pool(name="persist", bufs=1))
        sbuf = ctx.enter_context(tc.tile_pool(name="sbuf", bufs=4))
        rpool = ctx.enter_context(tc.tile_pool(name="rpool", bufs=4))
        trash_pool = ctx.enter_context(tc.tile_pool(name="trash", bufs=4))
        spsum = ctx.enter_context(tc.tile_pool(name="spsum", bufs=4, space="PSUM"))
        xtpsum = ctx.enter_context(tc.tile_pool(name="xtpsum", bufs=2, space="PSUM"))
        s2psum = ctx.enter_context(tc.tile_pool(name="s2psum", bufs=1, space="PSUM"))

        # ---- prologue ----
        ident = persist.tile([128, 128], f32)
        make_identity(nc, ident[:])

        zeros = persist.tile([128, M], f32)
        nc.gpsimd.memset(zeros[:], 0.0)

        negq_col = persist.tile([C, 1], f32)  # -0.25 for enorm from Wt
        nc.gpsimd.memset(negq_col[:], -0.25)
        neg_col = persist.tile([C, 1], f32)  # -1.0 for xnorm
        nc.gpsimd.memset(neg_col[:], -1.0)

        # embed natural -> PE transpose -> Wt = 2*embT [64, 512]
        e_nat = sbuf.tile([128, JT, C], f32, tag="xnat")
        nc.sync.dma_start(
            out=e_nat[:], in_=e_d.rearrange("(t p) c -> p t c", p=128)
        )
        eT_ps = xtpsum.tile([C + 1, CS], f32, tag="xT")
        for t in range(JT):
            nc.tensor.transpose(
                out=eT_ps[0:C, t * 128 : (t + 1) * 128],
                in_=e_nat[:, t, :],
                identity=ident[:],
            )
        Wt = persist.tile([C + 1, CS], f32)
        nc.scalar.activation(Wt[0:C, :], eT_ps[0:C, :], AF.Copy, scale=2.0)
        nc.gpsimd.memset(Wt[C : C + 1, :], 1.0)

        # neg_enorm row [1, 512] = -0.25 * sum_c (2*embT)^2
        sq_emb = sbuf.tile([C, CS], f32, tag="sq")
        nc.gpsimd.tensor_mul(sq_emb[:], Wt[0:C, :], Wt[0:C, :])
        en_ps = s2psum.tile([1, CS], f32, tag="W")
        nc.tensor.matmul(
            out=en_ps[:], lhsT=negq_col[:], rhs=sq_emb[:], start=True, stop=True
        )
        nen_row = persist.tile([1, CS], f32)
        nc.scalar.activation(nen_row[:], en_ps[:], AF.Copy)

        # neg_en_cols [128, JT]: [p, t] = -enorm[t*128+p]
        nen_cols = persist.tile([128, JT], f32)
        for t in range(JT):
            ec_ps = s2psum.tile([128, 1], f32, tag="WM")
            nc.tensor.transpose(
                out=ec_ps[:],
                in_=nen_row[0:1, t * 128 : (t + 1) * 128],
                identity=ident[0:1, 0:1],
            )
            nc.vector.tensor_copy(nen_cols[:, t : t + 1], ec_ps[:])

        # per-(jtile) accumulators over frames: V (vmax_s) and MI (m*) [128, nf]
        V_t = [
            persist.tile([128, nf], f32, tag=f"V{t}", name=f"V{t}") for t in range(JT)
        ]
        MI_t = [
            persist.tile([128, nf], f32, tag=f"MI{t}", name=f"MI{t}") for t in range(JT)
        ]

        # ---- main loop ----
        for f in range(nf):
            x_nat = sbuf.tile([128, JT, C], f32, tag="xnat")
            nc.sync.dma_start(
                out=x_nat[:], in_=x_d[f, :, :].rearrange("(t p) c -> p t c", p=128)
            )
            xT_ps = xtpsum.tile([C + 1, M], f32, tag="xT")
            for t in range(JT):
                nc.tensor.transpose(
                    out=xT_ps[0:C, t * 128 : (t + 1) * 128],
                    in_=x_nat[:, t, :],
                    identity=ident[:],
                )
            # sq = xT^2 (ACT, PSUM src); row 64 = -xnorm via (-1s) matmul into
            # the same PSUM bank; one ACT copy moves rows 0..64 to SBUF.
            sq = sbuf.tile([C, M], f32, tag="sq")
            nc.scalar.activation(sq[:], xT_ps[0:C, :], AF.Square)
            nc.tensor.matmul(
                out=xT_ps[C : C + 1, :], lhsT=neg_col[:], rhs=sq[:], start=True, stop=True
            )
            xT = sbuf.tile([C + 1, M], f32, tag="xTsb")
            nc.scalar.activation(xT[:], xT_ps[:], AF.Copy)

            for t in range(JT):
                s_ps = spsum.tile([128, M], f32, tag="s")
                nc.tensor.matmul(
                    out=s_ps[:],
                    lhsT=Wt[:, t * 128 : (t + 1) * 128],
                    rhs=xT[:],
                    start=True,
                    stop=True,
                )
                r = rpool.tile([128, M], f32, tag="r")
                nc.vector.tensor_tensor_scan(
                    out=r[:],
                    data0=s_ps[:],
                    data1=zeros[:],
                    initial=FLT_MIN,
                    op0=mybir.AluOpType.max,
                    op1=mybir.AluOpType.bypass,
                )
                # vmax_s = r[:, M-1] + neg_enorm_col (DVE for ACT-count tiles,
                # ACT for GPS-count tiles — engine balance)
                nc.gpsimd.tensor_add(
                    V_t[t][:, f : f + 1], r[:, M - 1 : M], nen_cols[:, t : t + 1]
                )
                # m* = #{r < vmax_g}: ACT Sign-count for 3 jtiles, DVE
                # is_lt-count (2x tensor_scalar) for the last (engine balance).
                if t < 3:
                    trash = trash_pool.tile([128, M], f32, tag="trashA")
                    nc.scalar.activation(
                        trash[:],
                        r[:],
                        AF.Sign,
                        bias=r[:, M - 1 : M],
                        scale=-1.0,
                        accum_out=MI_t[t][:, f : f + 1],
                    )
                else:
                    trash = trash_pool.tile([128, M], f32, tag="trashD")
                    nc.vector.tensor_scalar(
                        out=trash[:],
                        in0=r[:],
                        scalar1=r[:, M - 1 : M],
                        scalar2=None,
                        op0=mybir.AluOpType.is_lt,
                        op1=mybir.AluOpType.add,
                        accum_out=MI_t[t][:, f : f + 1],
                    )

        # ---- stage 2 ----
        W_ps = s2psum.tile([nf, CS], f32, tag="W")
        for t in range(JT):
            nc.tensor.transpose(
                out=W_ps[:, t * 128 : (t + 1) * 128], in_=V_t[t][:], identity=ident[:]
            )

        rW = persist.tile([nf, CS], f32)
        nc.vector.tensor_tensor_scan(
            out=rW[:],
            data0=W_ps[:],
            data1=zeros[0:nf, :],
            initial=FLT_MIN,
            op0=mybir.AluOpType.max,
            op1=mybir.AluOpType.bypass,
        )
        jstar_f = persist.tile([nf, 1], f32)
        trash2 = persist.tile([nf, CS], f32)
        nc.scalar.activation(
            trash2[:],
            rW[:],
            AF.Sign,
            bias=rW[:, CS - 1 : CS],
            scale=-1.0,
            accum_out=jstar_f[:],
        )

        # WM transposes reuse the same PSUM slot (tag "W") after W is done
        WM_ps = s2psum.tile([nf, CS], f32, tag="WM", name="WM_ps")
        for t in range(JT):
            nc.tensor.transpose(
                out=WM_ps[:, t * 128 : (t + 1) * 128], in_=MI_t[t][:], identity=ident[:]
            )

        # WM -> SBUF (fp32) for per-partition gather; also int32 for output
        WMs = persist.tile([128, CS], f32)
        nc.gpsimd.memset(WMs[:], 0.0)
        nc.scalar.activation(WMs[0:nf, :], WM_ps[:], AF.Copy)
        WM_i = persist.tile([nf, CS], i32)
        nc.vector.tensor_copy(WM_i[:], WMs[0:nf, :])
        nc.sync.dma_start(out=am_d[:], in_=WM_i[:])

        jstar_i = persist.tile([nf, 1], i32)
        nc.vector.tensor_copy(jstar_i[:], jstar_f[:])
        nc.sync.dma_start(out=ac_d[:], in_=jstar_i[:])

        # sel[f] = WMs[f, jstar[f]] via gpsimd indirect copy (128 parts; each
        # 16-partition group shares one 16-entry index list -> gather 16 and
        # mask-select entry p%16).
        jstar_u = persist.tile([128, 1], u16)
        nc.gpsimd.memset(jstar_u[:], 0)
        nc.vector.tensor_copy(jstar_u[0:nf, :], jstar_f[:])
        sel16 = persist.tile([128, 16], f32)
        nc.gpsimd.indirect_copy(sel16[:], WMs[:], jstar_u[:], True)

        ii16 = persist.tile([128, 16], i32)
        nc.gpsimd.iota(ii16[:], pattern=[[1, 16]], base=0, channel_multiplier=0)
        pp = persist.tile([128, 1], i32)
        nc.gpsimd.iota(pp[:], pattern=[[0, 1]], base=0, channel_multiplier=1)
        pdiv = persist.tile([128, 1], i32)
        nc.vector.tensor_scalar(
            out=pdiv[:],
            in0=pp[:],
            scalar1=4,
            scalar2=None,
            op0=mybir.AluOpType.arith_shift_right,
        )
        pdm = persist.tile([128, 1], i32)
        nc.vector.tensor_scalar(
            out=pdm[:], in0=pdiv[:], scalar1=16, scalar2=None, op0=mybir.AluOpType.mult
        )
        pm = persist.tile([128, 1], i32)
        nc.vector.tensor_sub(pm[:], pp[:], pdm[:])
        pmf = persist.tile([128, 1], f32)
        nc.vector.tensor_copy(pmf[:], pm[:])
        ii16f = persist.tile([128, 16], f32)
        nc.vector.tensor_copy(ii16f[:], ii16[:])
        mask16 = persist.tile([128, 16], f32)
        nc.vector.tensor_scalar(
            out=mask16[:],
            in0=ii16f[:],
            scalar1=pmf[:],
            scalar2=None,
            op0=mybir.AluOpType.is_equal,
        )
        sel16m = persist.tile([128, 16], f32)
        nc.vector.tensor_mul(sel16m[:], sel16[:], mask16[:])
        sel_f = persist.tile([128, 1], f32)
        nc.vector.tensor_reduce(
            sel_f[:], sel16m[:], axis=mybir.AxisListType.X, op=mybir.AluOpType.add
        )

        # offsets: xoff[f] = f*M + sel[f]
        sel_i = persist.tile([nf, 1], i32)
        nc.vector.tensor_copy(sel_i[:], sel_f[0:nf, :])
        fidx = persist.tile([nf, 1], i32)
        nc.gpsimd.iota(fidx[:], pattern=[[0, 1]], base=0, channel_multiplier=1)
        xoff = persist.tile([nf, 1], i32)
        nc.vector.tensor_scalar_mul(xoff[:], fidx[:], M)
        nc.vector.tensor_add(xoff[:], xoff[:], sel_i[:])

        # gathers
        xg = persist.tile([nf, C], f32)
        x_flat = x_d.rearrange("a b c -> (a b) c")
        nc.gpsimd.indirect_dma_start(
            out=xg[:],
            out_offset=None,
            in_=x_flat[:],
            in_offset=bass.IndirectOffsetOnAxis(ap=xoff[:, :1], axis=0),
        )
        eg = persist.tile([nf, C], f32)
        nc.gpsimd.indirect_dma_start(
            out=eg[:],
            out_offset=None,
            in_=e_d[:],
            in_offset=bass.IndirectOffsetOnAxis(ap=jstar_i[:, :1], axis=0),
        )
        res_sb = persist.tile([nf, C], f32)
        nc.vector.tensor_sub(res_sb[:], xg[:], eg[:])
        nc.sync.dma_start(out=res_d[:], in_=res_sb[:])


def build(nf=NF, num_devices=NCORES):
    from concourse import bacc
    import concourse.mybir as mybir
    import concourse.tile as tile

    nc = bacc.Bacc(
        "TRN2",
        target_bir_lowering=False,
        debug=False,
        num_devices=num_devices,
    )
    x_d = nc.dram_tensor("x", [nf, M, C], mybir.dt.float32, kind="ExternalInput").ap()
    e_d = nc.dram_tensor("embed", [CS, C], mybir.dt.float32, kind="ExternalInput").ap()
    am_d = nc.dram_tensor("argmin_prev", [nf, CS], mybir.dt.int32, kind="ExternalOutput").ap()
    ac_d = nc.dram_tensor("argmin_curr", [nf, 1], mybir.dt.int32, kind="ExternalOutput").ap()
    res_d = nc.dram_tensor("residual", [nf, C], mybir.dt.float32, kind="ExternalOutput").ap()
    with tile.TileContext(nc) as tc:
        _emit(tc, nf, x_d, e_d, am_d, ac_d, res_d)
    nc.compile()
    return nc


def _get_nc():
    if "nc" not in _cache:
        _cache["nc"] = build()
    return _cache["nc"]


def kernel(x, embed, argmin, last=None, **_ignored):
    from concourse import bass_utils

    x = np.ascontiguousarray(np.asarray(x), dtype=np.float32)
    embed = np.ascontiguousarray(np.asarray(embed), dtype=np.float32)
    argmin = np.asarray(argmin)

    nc = _get_nc()
    in_maps = [{"x": x[c * NF : (c + 1) * NF], "embed": embed} for c in range(NCORES)]
    res = bass_utils.run_bass_kernel_spmd(nc, in_maps, core_ids=list(range(NCORES)))
    _cache["last_result"] = res

    am = np.concatenate([r["argmin_prev"] for r in res.results], axis=0)  # (N, CS)
    ac = np.concatenate([r["argmin_curr"] for r in res.results], axis=0)  # (N, 1)
    rs = np.concatenate([r["residual"] for r in res.results], axis=0)  # (N, C)

    idx_dt = (
        argmin.dtype
        if argmin.dtype in (np.dtype(np.int32), np.dtype(np.int64))
        else np.dtype(np.int32)
    )
    argmin_out = am[None].astype(argmin.dtype, copy=False)
    argmin_curr = ac.astype(idx_dt, copy=False)
    residual = rs[:, None, :].astype(np.float32, copy=False)
    return residual, argmin_curr, argmin_out


# revision 36
# speedup vs baseline: 1.0489x; 1.0489x over previous
"""EuclideanCodebook (vq_codebook) Trainium2 Bass kernel.

Sharding: frames (N axis) split 8 ways; codebook replicated.

Per core (nf frames):
  prologue: embT via PE transpose; Wt = 2*embT; neg_enorm row/cols.
  For each frame f:
    DMA x_nat [128, 4, 64]; 4x PE transpose -> xT_ps [65, 512] PSUM rows 0..63
    ACT Square(xT_ps) -> sq_sb; PE (-1s) @ sq_sb -> xT_ps row 64 (= -xnorm)
    ACT Copy xT_ps[0:65] -> xT_sb
    for t in 4 j-tiles:
      PE matmul: g[j, m] = 2*e[j,:].x[m,:] - xnorm[m]   (K=65)
      DVE ttscan: r[j, m] = running max over m
      DVE: vmax_s[j] = r[:, M-1] + neg_enorm[j]  -> V_t[:, f]
      ACT Sign+accum: m*[j] = #{m : r[j,m] < r[j,M-1]} -> MI_t[:, f]
  Stage 2: PE-transpose V/MI to [frame, j], scan+count over j -> argmin_curr,
  indirect-DMA gathers for residual = x[f, sel] - embed[j*].
"""

from contextlib import ExitStack

import numpy as np

NCORES = 8
N, M, C, CS = 512, 512, 64, 512
NF = N // NCORES  # frames per core
JT = CS // 128    # j tiles

FLT_MIN = -3.0e38

_cache = {}


def _emit(tc, nf, x_d, e_d, am_d, ac_d, res_d):
    import concourse.bass as bass
    import concourse.mybir as mybir
    from concourse.masks import make_identity

    nc = tc.nc
    f32 = mybir.dt.float32
    i32 = mybir.dt.int32
    u16 = mybir.dt.uint16
    AF = mybir.ActivationFunctionType

    with ExitStack() as ctx:
        persist = ctx.enter_context(tc.tile_pool(name="persist", bufs=1))
        sbuf = ctx.enter_context(tc.tile_pool(name="sbuf", bufs=4))
        rpool = ctx.enter_context(tc.tile_pool(name="rpool", bufs=4))
        trash_pool = ctx.enter_context(tc.tile_pool(name="trash", bufs=4))
        spsum = ctx.enter_context(tc.tile_pool(name="spsum", bufs=4, space="PSUM"))
        xtpsum = ctx.enter_context(tc.tile_pool(name="xtpsum", bufs=2, space="PSUM"))
        s2psum = ctx.enter_context(tc.tile_pool(name="s2psum", bufs=1, space="PSUM"))

        # ---- prologue ----
        ident = persist.tile([128, 128], f32)
        make_identity(nc, ident[:])

        zeros = persist.tile([128, M], f32)
        nc.gpsimd.memset(zeros[:], 0.0)

        negq_col = persist.tile([C, 1], f32)  # -0.25 for enorm from Wt
        nc.gpsimd.memset(negq_col[:], -0.25)
        neg_col = persist.tile([C, 1], f32)  # -1.0 for xnorm
        nc.gpsimd.memset(neg_col[:], -1.0)

        # embed natural -> PE transpose -> Wt = 2*embT [64, 512]
        e_nat = sbuf.tile([128, JT, C], f32, tag="xnat")
        nc.sync.dma_start(
            out=e_nat[:], in_=e_d.rearrange("(t p) c -> p t c", p=128)
        )
        eT_ps = xtpsum.tile([C + 1, CS], f32, tag="xT")
        for t in range(JT):
            nc.tensor.transpose(
                out=eT_ps[0:C, t * 128 : (t + 1) * 128],
                in_=e_nat[:, t, :],
                identity=ident[:],
            )
        Wt = persist.tile([C + 1, CS], f32)
        nc.scalar.activation(Wt[0:C, :], eT_ps[0:C, :], AF.Copy, scale=2.0)
        nc.gpsimd.memset(Wt[C : C + 1, :], 1.0)

        # neg_enorm row [1, 512] = -0.25 * sum_c (2*embT)^2
        sq_emb = sbuf.tile([C, CS], f32, tag="sq")
        nc.gpsimd.tensor_mul(sq_emb[:], Wt[0:C, :], Wt[0:C, :])
        en_ps = s2psum.tile([1, CS], f32, tag="W")
        nc.tensor.matmul(
            out=en_ps[:], lhsT=negq_col[:], rhs=sq_emb[:], start=True, stop=True
        )
        nen_row = persist.tile([1, CS], f32)
        nc.scalar.activation(nen_row[:], en_ps[:], AF.Copy)

        # neg_en_cols [128, JT]: [p, t] = -enorm[t*128+p]
        nen_cols = persist.tile([128, JT], f32)
        for t in range(JT):
            ec_ps = s2psum.tile([128, 1], f32, tag="WM")
            nc.tensor.transpose(
                out=ec_ps[:],
                in_=nen_row[0:1, t * 128 : (t + 1) * 128],
                identity=ident[0:1, 0:1],
            )
            nc.vector.tensor_copy(nen_cols[:, t : t + 1], ec_ps[:])

        # per-(jtile) accumulators over frames: V (vmax_s) and MI (m*) [128, nf]
        V_t = [
            persist.tile([128, nf], f32, tag=f"V{t}", name=f"V{t}") for t in range(JT)
        ]
        MI_t = [
            persist.tile([128, nf], f32, tag=f"MI{t}", name=f"MI{t}") for t in range(JT)
        ]

        # ---- main loop ----
        for f in range(nf):
            x_nat = sbuf.tile([128, JT, C], f32, tag="xnat")
            nc.sync.dma_start(
                out=x_nat[:], in_=x_d[f, :, :].rearrange("(t p) c -> p t c", p=128)
            )
            xT_ps = xtpsum.tile([C + 1, M], f32, tag="xT")
            for t in range(JT):
                nc.tensor.transpose(
                    out=xT_ps[0:C, t * 128 : (t + 1) * 128],
                    in_=x_nat[:, t, :],
                    identity=ident[:],
                )
            # sq = xT^2 (ACT, PSUM src); row 64 = -xnorm via (-1s) matmul into
            # the same PSUM bank; one ACT copy moves rows 0..64 to SBUF.
            sq = sbuf.tile([C, M], f32, tag="sq")
            nc.scalar.activation(sq[:], xT_ps[0:C, :], AF.Square)
            nc.tensor.matmul(
                out=xT_ps[C : C + 1, :], lhsT=neg_col[:], rhs=sq[:], start=True, stop=True
            )
            xT = sbuf.tile([C + 1, M], f32, tag="xTsb")
            nc.scalar.activation(xT[:], xT_ps[:], AF.Copy)

            for t in range(JT):
                s_ps = spsum.tile([128, M], f32, tag="s")
                nc.tensor.matmul(
                    out=s_ps[:],
                    lhsT=Wt[:, t * 128 : (t + 1) * 128],
                    rhs=xT[:],
                    start=True,
                    stop=True,
                )
                r = rpool.tile([128, M], f32, tag="r")
                nc.vector.tensor_tensor_scan(
                    out=r[:],
                    data0=s_ps[:],
                    data1=zeros[:],
                    initial=FLT_MIN,
                    op0=mybir.AluOpType.max,
                    op1=mybir.AluOpType.bypass,
                )
                # vmax_s = r[:, M-1] + neg_enorm_col (DVE for ACT-count tiles,
                # ACT for GPS-count tiles — engine balance)
                nc.gpsimd.tensor_add(
                    V_t[t][:, f : f + 1], r[:, M - 1 : M], nen_cols[:, t : t + 1]
                )
                # m* = #{r < vmax_g}: ACT Sign-count for 2 jtiles, DVE
                # is_lt-count (2x tensor_scalar) for 2 (engine balance).
                if t < 2:
                    trash = trash_pool.tile([128, M], f32, tag="trashA")
                    nc.scalar.activation(
                        trash[:],
                        r[:],
                        AF.Sign,
                        bias=r[:, M - 1 : M],
                        scale=-1.0,
                        accum_out=MI_t[t][:, f : f + 1],
                    )
                else:
                    trash = trash_pool.tile([128, M], f32, tag="trashD")
                    nc.vector.tensor_scalar(
                        out=trash[:],
                        in0=r[:],
                        scalar1=r[:, M - 1 : M],
                        scalar2=None,
                        op0=mybir.AluOpType.is_lt,
                        op1=mybir.AluOpType.add,
                        accum_out=MI_t[t][:, f : f + 1],
                    )

        # ---- stage 2 ----
        W_ps = s2psum.tile([nf, CS], f32, tag="W")
        for t in range(JT):
            nc.tensor.transpose(
                out=W_ps[:, t * 128 : (t + 1) * 128], in_=V_t[t][:], identity=ident[:]
            )

        rW = persist.tile([nf, CS], f32)
        nc.vector.tensor_tensor_scan(
            out=rW[:],
            data0=W_ps[:],
            data1=zeros[0:nf, :],
            initial=FLT_MIN,
            op0=mybir.AluOpType.max,
            op1=mybir.AluOpType.bypass,
        )
        jstar_f = persist.tile([nf, 1], f32)
        trash2 = persist.tile([nf, CS], f32)
        nc.scalar.activation(
            trash2[:],
            rW[:],
            AF.Sign,
            bias=rW[:, CS - 1 : CS],
            scale=-1.0,
            accum_out=jstar_f[:],
        )

        # WM transposes reuse the same PSUM slot (tag "W") after W is done
        WM_ps = s2psum.tile([nf, CS], f32, tag="WM", name="WM_ps")
        for t in range(JT):
            nc.tensor.transpose(
                out=WM_ps[:, t * 128 : (t + 1) * 128], in_=MI_t[t][:], identity=ident[:]
            )

        # WM -> SBUF (fp32) for per-partition gather; also int32 for output
        WMs = persist.tile([128, CS], f32)
        nc.gpsimd.memset(WMs[:], 0.0)
        nc.scalar.activation(WMs[0:nf, :], WM_ps[:], AF.Copy)
        WM_i = persist.tile([nf, CS], i32)
        nc.vector.tensor_copy(WM_i[:], WMs[0:nf, :])
        nc.sync.dma_start(out=am_d[:], in_=WM_i[:])

        jstar_i = persist.tile([nf, 1], i32)
        nc.vector.tensor_copy(jstar_i[:], jstar_f[:])
        nc.sync.dma_start(out=ac_d[:], in_=jstar_i[:])

        # sel[f] = WMs[f, jstar[f]] via gpsimd indirect copy (128 parts; each
        # 16-partition group shares one 16-entry index list -> gather 16 and
        # mask-select entry p%16).
        jstar_u = persist.tile([128, 1], u16)
        nc.gpsimd.memset(jstar_u[:], 0)
        nc.vector.tensor_copy(jstar_u[0:nf, :], jstar_f[:])
        sel16 = persist.tile([128, 16], f32)
        nc.gpsimd.indirect_copy(sel16[:], WMs[:], jstar_u[:], True)

        ii16 = persist.tile([128, 16], i32)
        nc.gpsimd.iota(ii16[:], pattern=[[1, 16]], base=0, channel_multiplier=0)
        pp = persist.tile([128, 1], i32)
        nc.gpsimd.iota(pp[:], pattern=[[0, 1]], base=0, channel_multiplier=1)
        pdiv = persist.tile([128, 1], i32)
        nc.vector.tensor_scalar(
            out=pdiv[:],
            in0=pp[:],
            scalar1=4,
            scalar2=None,
            op0=mybir.AluOpType.arith_shift_right,
        )
        pdm = persist.tile([128, 1], i32)
        nc.vector.tensor_scalar(
            out=pdm[:], in0=pdiv[:], scalar1=16, scalar2=None, op0=mybir.AluOpType.mult
        )
        pm = persist.tile([128, 1], i32)
        nc.vector.tensor_sub(pm[:], pp[:], pdm[:])
        pmf = persist.tile([128, 1], f32)
        nc.vector.tensor_copy(pmf[:], pm[:])
        ii16f = persist.tile([128, 16], f32)
        nc.vector.tensor_copy(ii16f[:], ii16[:])
        mask16 = persist.tile([128, 16], f32)
        nc.vector.tensor_scalar(
            out=mask16[:],
            in0=ii16f[:],
            scalar1=pmf[:],
            scalar2=None,
            op0=mybir.AluOpType.is_equal,
        )
        sel16m = persist.tile([128, 16], f32)
        nc.vector.tensor_mul(sel16m[:], sel16[:], mask16[:])
        sel_f = persist.tile([128, 1], f32)
        nc.vector.tensor_reduce(
            sel_f[:], sel16m[:], axis=mybir.AxisListType.X, op=mybir.AluOpType.add
        )

        # offsets: xoff[f] = f*M + sel[f]
        sel_i = persist.tile([nf, 1], i32)
        nc.vector.tensor_copy(sel_i[:], sel_f[0:nf, :])
        fidx = persist.tile([nf, 1], i32)
        nc.gpsimd.iota(fidx[:], pattern=[[0, 1]], base=0, channel_multiplier=1)
        xoff = persist.tile([nf, 1], i32)
        nc.vector.tensor_scalar_mul(xoff[:], fidx[:], M)
        nc.vector.tensor_add(xoff[:], xoff[:], sel_i[:])

        # gathers
        xg = persist.tile([nf, C], f32)
        x_flat = x_d.rearrange("a b c -> (a b) c")
        nc.gpsimd.indirect_dma_start(
            out=xg[:],
            out_offset=None,
            in_=x_flat[:],
            in_offset=bass.IndirectOffsetOnAxis(ap=xoff[:, :1], axis=0),
        )
        eg = persist.tile([nf, C], f32)
        nc.gpsimd.indirect_dma_start(
            out=eg[:],
            out_offset=None,
            in_=e_d[:],
            in_offset=bass.IndirectOffsetOnAxis(ap=jstar_i[:, :1], axis=0),
        )
        res_sb = persist.tile([nf, C], f32)
        nc.vector.tensor_sub(res_sb[:], xg[:], eg[:])
        nc.sync.dma_start(out=res_d[:], in_=res_sb[:])


def build(nf=NF, num_devices=NCORES):
    from concourse import bacc
    import concourse.mybir as mybir
    import concourse.tile as tile

    nc = bacc.Bacc(
        "TRN2",
        target_bir_lowering=False,
        debug=False,
        num_devices=num_devices,
    )
    x_d = nc.dram_tensor("x", [nf, M, C], mybir.dt.float32, kind="ExternalInput").ap()
    e_d = nc.dram_tensor("embed", [CS, C], mybir.dt.float32, kind="ExternalInput").ap()
    am_d = nc.dram_tensor("argmin_prev", [nf, CS], mybir.dt.int32, kind="ExternalOutput").ap()
    ac_d = nc.dram_tensor("argmin_curr", [nf, 1], mybir.dt.int32, kind="ExternalOutput").ap()
    res_d = nc.dram_tensor("residual", [nf, C], mybir.dt.float32, kind="ExternalOutput").ap()
    with tile.TileContext(nc) as tc:
        _emit(tc, nf, x_d, e_d, am_d, ac_d, res_d)
    nc.compile()
    return nc


def _get_nc():
    if "nc" not in _cache:
        _cache["nc"] = build()
    return _cache["nc"]


def kernel(x, embed, argmin, last=None, **_ignored):
    from concourse import bass_utils

    x = np.ascontiguousarray(np.asarray(x), dtype=np.float32)
    embed = np.ascontiguousarray(np.asarray(embed), dtype=np.float32)
    argmin = np.asarray(argmin)

    nc = _get_nc()
    in_maps = [{"x": x[c * NF : (c + 1) * NF], "embed": embed} for c in range(NCORES)]
    res = bass_utils.run_bass_kernel_spmd(nc, in_maps, core_ids=list(range(NCORES)))
    _cache["last_result"] = res

    am = np.concatenate([r["argmin_prev"] for r in res.results], axis=0)  # (N, CS)
    ac = np.concatenate([r["argmin_curr"] for r in res.results], axis=0)  # (N, 1)
    rs = np.concatenate([r["residual"] for r in res.results], axis=0)  # (N, C)

    idx_dt = (
        argmin.dtype
        if argmin.dtype in (np.dtype(np.int32), np.dtype(np.int64))
        else np.dtype(np.int32)
    )
    argmin_out = am[None].astype(argmin.dtype, copy=False)
    argmin_curr = ac.astype(idx_dt, copy=False)
    residual = rs[:, None, :].astype(np.float32, copy=False)
    return residual, argmin_curr, argmin_out
